# revision 15
# baseline (speedup 1.0000x reference)
"""Trainium2 Bass kernel for nn_Compressor (sparse_attention, hierarchical window MLP).

Reference computation (per batch b, head h):
  windows w=0..510 over k[b,h] (S=8192, D=128), window length 32, stride 16
  x[w, l, :] = k[16w+l, :] + pe[l, :]
  5 stages of pairwise-merge MLP: x <- silu(x.reshape(-1, 256) @ w_down[i].T)
  out[w+1] = x @ w_stop.T   ; out[0] = 0 (prepended zero window)

Sharding: head-parallel across 8 cores (B*H = 32 -> 4 heads/core), weights
replicated, no cross-device comms.

Stage-0 sharing: adjacent row pairs (2t, 2t+1) are shared by two windows in
the same even/odd role, so Z[:, t] = W0_even @ kT[:, 2t] + W0_odd @ kT[:, 2t+1]
is computed once per pair; the window-position-dependent pe enters through the
ScalarE activation bias: s0[:, j, w] = silu(Z[:, 8w+j'] + W0 @ pe_pair_j).

Schedule: the core's 4 heads run in lockstep so each ScalarE activation covers
4*511 = 2044 columns (ACT is the bottleneck: 1 cycle/element at 1.2 GHz plus
~290 ns per instruction).  PSUM is two 4-bank quads; ALL work units (stage-0
Z, merge stages, stop) alternate between the quads in emission order, so every
unit's matmuls depend only on the activation two units back and ACT stays
saturated.  Warm-up matmuls before the first data arrives plus dummy
LDWEIGHTS after every unit keep the PE HAM clock gate at 2.4 GHz (PE work
at the cold 1.2 GHz clock cannot keep ahead of ACT).  k is pre-transposed on
the host into per-e-plane chunks (plain contiguous DMAs) streamed through a
2-buffer pool whose WAR semaphores also serialize the chunk transfers so no
chunk steals SDMA bandwidth from an earlier, more urgent one.
"""

import numpy as np

B, H, S, D = 2, 16, 8192, 128
BH = B * H
NCORES = 8
HPC = BH // NCORES  # heads per core = 4
NB = (S - 32) // 16 + 1  # 511 sliding windows
NW = NB + 1  # 512 output rows per head (incl. zero window)

_BASS_CACHE = {}


def _build_bass():
    import concourse.bacc as bacc
    import concourse.mybir as mybir
    import concourse.tile as tile
    from bass_rust import add_dep_helper

    f32 = mybir.dt.float32
    bf16 = mybir.dt.bfloat16
    SILU = mybir.ActivationFunctionType.Silu

    nc = bacc.Bacc()
    # k5[j, d, e2, h, parity, c] = k[head h, row 16c + 2*(2j+e2) + parity, d]
    # (bf16, e-pair-major so one chunk DMA has 16KB-contiguous partition runs)
    k5 = nc.dram_tensor("k5", [4, 128, 2, HPC, 2, 512], bf16, kind="ExternalInput")
    # wdt[d_in, i, half, o] = w_down[i][o, 128*half + d_in]  (host pre-laid-out)
    wdt = nc.dram_tensor("wdt", [128, 5, 2, 128], bf16, kind="ExternalInput")
    pe0 = nc.dram_tensor("pe0", [128, 16], f32, kind="ExternalInput")
    wst = nc.dram_tensor("wst", [128, 128], bf16, kind="ExternalInput")
    # out_d[o, h, w] = out[head h, window w+1, feature o]  (bf16, host transposes)
    out_d = nc.dram_tensor("out_d", [128, HPC, NB], bf16, kind="ExternalOutput")

    with tile.TileContext(nc) as tc:
        with (
            tc.tile_pool(name="consts", bufs=1) as consts,
            tc.tile_pool(name="kqp", bufs=2) as kqp,
            tc.tile_pool(name="sbp", bufs=1) as sbp,
            tc.tile_pool(name="psA", bufs=1, space="PSUM") as psA,
            tc.tile_pool(name="psB", bufs=1, space="PSUM") as psB,
        ):
            wst_sb = consts.tile([128, 128], bf16, name="wst_sb")
            nc.gpsimd.dma_start(out=wst_sb, in_=wst[:])
            pe0_sb = consts.tile([128, 16], f32, name="pe0_sb")
            nc.gpsimd.dma_start(out=pe0_sb, in_=pe0[:])
            wd_sb = consts.tile([128, 5, 2, 128], bf16, name="wd_sb")
            nc.gpsimd.dma_start(out=wd_sb, in_=wdt[:])


            # two 4-bank PSUM quads (bank = one head), alternated across units
            regions = [
                psA.tile([128, HPC, 512], f32, name="zq"),
                psB.tile([128, HPC, 512], f32, name="sp"),
            ]

            # SBUF stage outputs, all [128, planes, heads, windows] bf16
            s0 = sbp.tile([128, 16, HPC, NB], bf16, name="s0")
            s1 = sbp.tile([128, 8, HPC, NB], bf16, name="s1")
            s2 = sbp.tile([128, 4, HPC, NB], bf16, name="s2")
            s3 = sbp.tile([128, 2, HPC, NB], bf16, name="s3")
            s4 = sbp.tile([128, HPC, NB], bf16, name="s4")
            out_sb = sbp.tile([128, HPC, NB], bf16, name="out_sb")

            # pin the PE and ACT instruction streams to emission order:
            # the tile scheduler otherwise reorders them and a matmul
            # waiting on a late DMA can head-of-line-block ready work
            order = {"mm": None, "act": None}

            def MM(out, **kw):
                m = nc.tensor.matmul(out, **kw)
                if order["mm"] is not None:
                    add_dep_helper(m.ins, order["mm"].ins, False, "pe order")
                order["mm"] = m
                return m

            def ACT(**kw):
                a = nc.scalar.activation(**kw)
                if order["act"] is not None:
                    add_dep_helper(a.ins, order["act"].ins, False, "act order")
                order["act"] = a
                return a

            kts = {}

            def dma_j(j, split=False):
                kq = kqp.tile([128, 2, HPC, 2, 512], bf16, name="kq")
                if split:
                    dmas = [
                        nc.sync.dma_start(out=kq[:, 0], in_=k5[j, :, 0]),
                        nc.sync.dma_start(out=kq[:, 1], in_=k5[j, :, 1]),
                    ]
                else:
                    dmas = [nc.sync.dma_start(out=kq, in_=k5[j])]
                kts[2 * j] = kts[2 * j + 1] = kq
                return dmas

            def mm_group(reg, st, h0, rhs_of, n=512, dup=False, nh=2):
                first = None
                # dup=True runs the accumulation group twice: the first pass
                # is thrown away (overwritten by the second), but keeps the
                # PE matmul stream dense so the HAM clock gate stays open.
                # par-major order lets the start=True half run before the
                # odd-parity operand's producer activation has finished.
                for _ in range(2 if dup else 1):
                    for par in range(2):
                        for h in range(h0, h0 + nh):
                            mm = MM(
                                reg[:, h, 0:n], lhsT=wd_sb[:, st, par, :],
                                rhs=rhs_of(h, par),
                                start=(par == 0), stop=(par == 1),
                            )
                            first = first or mm
                return first

            def z_unit(e, reg, split_act=False, dup=False):
                kq = kts.pop(e)
                first = None

                def acts(a, b):
                    ACT(
                        out=s0[:, e, a:b, :], in_=reg[:, a:b, 0:NB], func=SILU,
                        bias=pe0_sb[:, e : e + 1], scale=1.0,
                    )
                    ACT(
                        out=s0[:, e + 8, a:b, :], in_=reg[:, a:b, 1 : NB + 1],
                        func=SILU, bias=pe0_sb[:, e + 8 : e + 9], scale=1.0,
                    )

                if split_act:
                    for h0 in (0, 2):
                        mm = mm_group(reg, 0, h0,
                                      lambda h, par: kq[:, e % 2, h, par, :],
                                      dup=dup and h0 == 0)
                        first = first or mm
                        acts(h0, h0 + 2)
                else:
                    first = mm_group(reg, 0, 0,
                                     lambda h, par: kq[:, e % 2, h, par, :],
                                     dup=dup, nh=4)
                    acts(0, 4)
                return first

            def stage_unit(st, p, prev, cur, reg, dup=False, split_act=False):
                mm_group(reg, st, 0,
                         lambda h, par: prev[:, 2 * p + par, h, :], n=NB,
                         dup=dup, nh=4)
                halves = ((0, 2), (2, 4)) if split_act else ((0, 4),)
                for a, b in halves:
                    ACT(
                        out=cur[:, p, a:b, :], in_=reg[:, a:b, 0:NB], func=SILU,
                    )

            # chunk j0 covers e0 (ungated, races only the small const DMAs;
            # starting Z0 cold-but-early beats warming PE first) and e1
            # (gated on Z0's first matmul).  Chunk j1 is also gated on Z0;
            # j2/j3 are serialized by the 2-buffer pool's WAR semaphores.
            # The 16KB-contiguous partition runs of full-pair chunks double
            # the SDMA descriptor size (per-descriptor overhead dominates
            # below ~8KB).
            d_e0, d_e1 = dma_j(0, split=True)
            zmm0 = z_unit(0, regions[0], split_act=True, dup=False)
            add_dep_helper(d_e1.ins, zmm0.ins, True, "stagger chunk e1")
            d_j1 = dma_j(1)[0]
            add_dep_helper(d_j1.ins, zmm0.ins, True, "stagger chunk j1")

            # unit list: every unit's data+region deps are >=2 units back
            units = [("z", 1), ("s", 1, 0), ("z", 2), ("s", 1, 4), ("z", 3),
                     ("s", 1, 1), ("z", 4), ("s", 1, 5), ("z", 5), ("s", 1, 2),
                     ("z", 6), ("s", 1, 6), ("z", 7), ("s", 1, 3), ("s", 1, 7),
                     ("s", 2, 0), ("s", 2, 1), ("s", 2, 2), ("s", 2, 3),
                     ("s", 3, 0), ("s", 3, 1)]
            sin = {1: (s0, s1), 2: (s1, s2), 3: (s2, s3)}
            for n, u in enumerate(units):
                reg = regions[(n + 1) % 2]
                if u[0] == "z":
                    e = u[1]
                    z_unit(e, reg)
                    if e in (3, 5):
                        dma_j(e // 2 + 1)
                else:
                    _, st, p = u
                    prev, cur = sin[st]
                    stage_unit(st, p, prev, cur, reg, dup=(st == 1),
                               split_act=(st == 3))

            # stage 4 (single plane), half-split ACTs so the stop pipeline
            # drains per head-pair
            n = len(units) + 1
            reg4 = regions[n % 2]
            mm_group(reg4, 4, 0, lambda h, par: s3[:, par, h, :], n=NB, nh=4)
            regS = regions[(n + 1) % 2]
            for h0 in (0, 2):
                ACT(
                    out=s4[:, h0 : h0 + 2, :], in_=reg4[:, h0 : h0 + 2, 0:NB],
                    func=SILU,
                )
            # w_stop, weight-stationary: out[o, w] in PSUM bank h of regS;
            # copies split across ScalarE (copy is in the resident
            # silu_and_others table set - no table reload) and VectorE;
            # output DMAs via sync (HWDGE) rather than gpsimd (SWDGE)
            for h in range(HPC):
                MM(
                    regS[:, h, 0:NB], lhsT=wst_sb, rhs=s4[:, h, :],
                    start=True, stop=True,
                )
            for h0 in (0, 2):
                cp = nc.scalar.copy(out=out_sb[:, h0, :], in_=regS[:, h0, 0:NB])
                add_dep_helper(cp.ins, order["act"].ins, False, "act order")
                order["act"] = cp
                nc.vector.tensor_copy(
                    out=out_sb[:, h0 + 1, :], in_=regS[:, h0 + 1, 0:NB]
                )
                nc.sync.dma_start(
                    out=out_d[:, h0 : h0 + 2, :], in_=out_sb[:, h0 : h0 + 2, :]
                )

    if not nc.is_finalized():
        nc.finalize()
    return nc


def _prep_host_inputs(k, pe, w_down, w_stop):
    import ml_dtypes

    bf16 = ml_dtypes.bfloat16
    k = np.asarray(k, dtype=np.float32)
    pe = np.asarray(pe, dtype=np.float32)
    w_down = np.asarray(w_down, dtype=np.float32)
    w_stop = np.asarray(w_stop, dtype=np.float32)

    # k5[core, j, d, e2, h, parity, c] =
    #     k[head 4*core+h, row 16c + 2*(2j+e2) + parity, d]
    k4 = k.reshape(NCORES, HPC, 512, 4, 2, 2, 128).astype(bf16)
    k5 = np.ascontiguousarray(k4.transpose(0, 3, 6, 4, 1, 5, 2))
    # wdt[d_in, i, half, o] = w_down[i][o, 128*half + d_in]
    wdt = np.ascontiguousarray(
        w_down.transpose(0, 2, 1).reshape(5, 2, 128, 128).transpose(2, 0, 1, 3)
    ).astype(bf16)
    # pe0[o, j] = sum_i w_down[0][o, i] * concat(pe[2j], pe[2j+1])[i]
    pe_pairs = pe.reshape(16, 256).astype(np.float64)
    pe0 = (w_down[0].astype(np.float64) @ pe_pairs.T).astype(np.float32)
    wst = np.ascontiguousarray(w_stop.T).astype(bf16)
    return k5, wdt, pe0, wst


def run(k, pe, w_down, w_stop, trace=False, trace_kwargs=None):
    from concourse.bass_utils import run_bass_kernel_spmd

    k5, wdt, pe0, wst = _prep_host_inputs(k, pe, w_down, w_stop)

    if "nc" not in _BASS_CACHE:
        _BASS_CACHE["nc"] = _build_bass()
    nc = _BASS_CACHE["nc"]

    in_maps = [
        {
            "k5": np.ascontiguousarray(k5[c]),
            "wdt": wdt,
            "pe0": pe0,
            "wst": wst,
        }
        for c in range(NCORES)
    ]
    res = run_bass_kernel_spmd(
        nc, in_maps, core_ids=list(range(NCORES)), trace=trace,
        **(trace_kwargs or {}),
    )
    out = np.empty((BH, NW, D), dtype=np.float32)
    for c in range(NCORES):
        r = np.asarray(res.results[c]["out_d"], dtype=np.float32)  # [o, h, w]
        for hh in range(HPC):
            row = HPC * c + hh
            out[row, 0, :] = 0.0
            out[row, 1:, :] = r[:, hh, :].T
    out = out.reshape(B, H, NW, D)
    return out, res


def kernel(k, pe, w_down, w_stop):
    out, _ = run(k, pe, w_down, w_stop, trace=False)
    return out
